# revision 1
# baseline (speedup 1.0000x reference)
"""LocallyConnected1d Bass kernel for 8 TRN2 NeuronCores.

Problem: x [64, 64, 512] f32, weight [1, 64, 64, 504, 9] f32
         out[b, o, l] = sum_{i,k} x[b, i, l+k] * weight[0, o, i, l, k]
L_out = 504 is sharded 8 x 63; inputs are laid out on the host (bf16)
and the full result is gathered and cast back to f32.

Design (per core):
  - Block-diagonal position pairing: positions (2g, 2g+1) of a column
    group share 9 matmuls of 64-deep contraction.  The stationary
    tile_q [128,128] holds x col q in rows 0:64 x cols 0:64 and x col
    q+1 in rows 64:128 x cols 64:128 (zeros off-diagonal; shipping the
    zeros densely beats a strided data-only DMA, whose <512 B runs pay
    a 2x descriptor penalty); the moving rhs stacks w[2g, k=p] over
    w[2g+1, k=p]; one matmul writes the FULL [128, 64] group.  That is
    576 moving columns per two positions instead of 640 -- the exact
    576/128 = 4.5 accumulation passes/position this contraction allows.
  - l=62 (group 31 has no partner) runs the classic 5-pass scheme
    MID-STREAM so the kernel tail only drains one bank.
  - Outputs accumulate across 7 PSUM banks; each bank is drained by one
    bulk cast-copy (f32 -> bf16) the moment it closes, overlapping the
    next bank's matmuls.  Tail: penultimate bank drains via the
    pre-warmed ACT engine, the last bank's [128,64] DVE copy is
    uncontended, and one merged SP DMA ships the final two groups.
  - Input DMAs are split across the three DMA rings (SP / ACT / Pool)
    in PE-consumption order with minimum-size first chunks, so the PE
    starts at the first-input latency floor and runs gap-free.
"""

import numpy as np
import ml_dtypes

B = 64
CI = 64
CO = 64
K = 9
L = 512
L_OUT = 504
N_CORES = 8
LP = L_OUT // N_CORES          # 63 positions per core
HALO = LP + K - 1              # 71 x-columns per core
NGRP = 32                      # column groups (l//2); g31 = l62 alone
NQ = 70                        # block-diag stationary tiles q = 0..69
BANK_G = [(0, 7), (7, 14), (14, 21), (21, 28), (28, 30), (30, 31), (31, 32)]
NBANK = len(BANK_G)

W9CHUNKS = [(0, 1), (1, 3), (3, 6), (6, 10), (10, 14), (14, 18), (18, 22),
            (22, 26), (26, 31)]
XBCHUNKS = [(0, 5), (5, 9), (9, 15), (15, 23), (23, 33), (33, 45), (45, 57),
            (57, 70)]

OUT_COLS = NGRP * CO           # 2048 bf16 cols

SP, ACT, POOL = "sync", "scalar", "gpsimd"
# chunks round-robined across rings in PE-consumption order; ring firsts
# carry the PE-start gates (xb0 + w9_0 on HWDGE, xb1 on Pool)
DMA_PLAN = {
    SP: [("xb", 0), ("w9", 1), ("w62", 0), ("xb", 3), ("w9", 4), ("xb", 6),
         ("w9", 7)],
    ACT: [("w9", 0), ("xb", 2), ("w9", 3), ("xb", 5), ("w9", 6), ("w9", 8)],
    POOL: [("xb", 1), ("x62", 0), ("w9", 2), ("xb", 4), ("w9", 5), ("xb", 7)],
}


def _build_bass():
    import concourse.bass as bass
    import concourse.mybir as mybir
    from concourse.tile import TileContext

    dt = mybir.dt.bfloat16
    nc = bass.Bass()

    xb_d = nc.dram_tensor("xb", [128, NQ * 128], dt, kind="ExternalInput")
    w9_d = nc.dram_tensor("w9", [128, 31 * K * CO], dt, kind="ExternalInput")
    x62_d = nc.dram_tensor("x62", [128, 5 * B], dt, kind="ExternalInput")
    w62_d = nc.dram_tensor("w62", [128, 5 * CO], dt, kind="ExternalInput")
    out_d = nc.dram_tensor("out", [128, OUT_COLS], dt, kind="ExternalOutput")

    with TileContext(nc) as tc:
        with (
            tc.tile_pool(name="xc", bufs=1) as xpool,
            tc.tile_pool(name="wc", bufs=1) as wpool,
            tc.tile_pool(name="ps", bufs=1, space="PSUM") as ppool,
            tc.tile_pool(name="ob", bufs=1) as opool,
        ):
            xbtiles = [xpool.tile([128, (e - s) * 128], dt, name=f"xb{c}")
                       for c, (s, e) in enumerate(XBCHUNKS)]
            w9tiles = [wpool.tile([128, (e - s) * K * CO], dt, name=f"w9{c}")
                       for c, (s, e) in enumerate(W9CHUNKS)]
            x62 = xpool.tile([128, 5 * B], dt, name="x62")
            w62 = wpool.tile([128, 5 * CO], dt, name="w62")

            def issue(ring, kind, i):
                eng = getattr(nc, ring)
                if kind == "w9":
                    s, e = W9CHUNKS[i]
                    eng.dma_start(out=w9tiles[i],
                                  in_=w9_d[:, s * K * CO:e * K * CO])
                elif kind == "xb":
                    s, e = XBCHUNKS[i]
                    eng.dma_start(out=xbtiles[i],
                                  in_=xb_d[:, s * 128:e * 128])
                elif kind == "x62":
                    eng.dma_start(out=x62, in_=x62_d[:, :])
                elif kind == "w62":
                    eng.dma_start(out=w62, in_=w62_d[:, :])

            maxlen = max(len(v) for v in DMA_PLAN.values())
            for j in range(maxlen):
                for ring in (POOL, SP, ACT):
                    if j < len(DMA_PLAN[ring]):
                        issue(ring, *DMA_PLAN[ring][j])

            out_sb = opool.tile([128, OUT_COLS], dt)
            psum = [ppool.tile([128, 512], mybir.dt.float32,
                               name=f"pb{t}") for t in range(NBANK)]
            # l=63 does not exist; the final [64,64] copy leaves the bottom
            # half of the g31 strip unwritten -- zero it up front.
            nc.vector.memset(out_sb[64:128, 31 * CO:32 * CO], 0.0)
            # pre-warm ACT's lazy activation-table load off the tail path
            scratch = opool.tile([64, 1], mybir.dt.float32, name="preld")
            nc.scalar.copy(out=scratch, in_=w62[0:64, 0:1])

            def chunk_of(v, chunks):
                for c, (s, e) in enumerate(chunks):
                    if s <= v < e:
                        return c, s
                raise AssertionError

            def drain(bank):
                gs, ge = BANK_G[bank]
                lo, hi = gs * CO, ge * CO
                if bank == 4:
                    # penultimate bank on the pre-warmed ACT engine, with
                    # its own ACT out-DMA -- keeps DVE free for bank 5
                    nc.scalar.copy(out=out_sb[:, lo:hi],
                                   in_=psum[bank][:, :hi - lo])
                    nc.scalar.dma_start(out=out_d[:, lo:hi],
                                        in_=out_sb[:, lo:hi])
                elif bank == 5:
                    # last-closing bank: DVE copy (uncontended), then the
                    # merged final SP DMA ships g30+g31 (g31 drained early)
                    nc.vector.tensor_copy(
                        out=out_sb[:, lo:hi], in_=psum[bank][:, :hi - lo])
                    nc.sync.dma_start(out=out_d[:, 30 * CO:32 * CO],
                                      in_=out_sb[:, 30 * CO:32 * CO])
                elif bank == 6:
                    nc.vector.tensor_copy(
                        out=out_sb[0:64, lo:hi],
                        in_=psum[bank][0:64, :hi - lo])
                else:
                    nc.vector.tensor_copy(
                        out=out_sb[:, lo:hi], in_=psum[bank][:, :hi - lo])
                    ring = (nc.gpsimd, nc.gpsimd, nc.gpsimd, nc.gpsimd)[bank]
                    ring.dma_start(out=out_d[:, lo:hi],
                                   in_=out_sb[:, lo:hi])

            for g in range(31):
                bank, bs = chunk_of(g, BANK_G)
                slot = g - bs
                outp = psum[bank][:, slot * CO:(slot + 1) * CO]
                wc, wcs = chunk_of(g, W9CHUNKS)
                for p in range(K):
                    q = 2 * g + p
                    xc, xcs = chunk_of(q, XBCHUNKS)
                    lhsT = xbtiles[xc][:, (q - xcs) * 128:(q - xcs + 1) * 128]
                    off = ((g - wcs) * K + p) * CO
                    rhs = w9tiles[wc][:, off:off + CO]
                    nc.tensor.matmul(outp, lhsT, rhs,
                                     start=(p == 0), stop=(p == K - 1))
                if g == BANK_G[bank][1] - 1:
                    drain(bank)
                if g == 12:
                    # l = 62 (lone position of group 31, bank 6) runs
                    # mid-stream so the kernel tail only drains bank 5;
                    # classic 5-pass scheme, copy-only drain
                    outp62 = psum[6][0:64, 0:64]
                    for s in range(4):
                        nc.tensor.matmul(outp62, x62[:, s * B:(s + 1) * B],
                                         w62[:, s * CO:(s + 1) * CO],
                                         start=(s == 0), stop=False)
                    nc.tensor.matmul(outp62, x62[0:64, 4 * B:5 * B],
                                     w62[0:64, 4 * CO:5 * CO],
                                     start=False, stop=True)
                    drain(6)
    _split_multi_waits(nc, mybir)
    return nc


def _split_multi_waits(nc, mybir):
    """This walrus build encodes at most ONE sync wait per instruction;
    hoist extra waits onto single-wait NoOps (semantically identical)."""
    for f in nc.m.functions:
        for bb in f.blocks:
            out = []
            for inst in bb.instructions:
                si = inst.sync_info
                waits = list(si.on_wait) if si is not None and si.on_wait else []
                if len(waits) > 1:
                    for k, w in enumerate(waits[:-1]):
                        out.append(mybir.InstNoOp(
                            name=f"{inst.name}-wsplit{k}",
                            engine=inst.engine,
                            sync_info=mybir.SyncInfo(on_wait=[w], on_update=[]),
                            bass_nofuse=True))
                    inst.sync_info = mybir.SyncInfo(
                        on_wait=[waits[-1]],
                        on_update=list(si.on_update) if si.on_update else [])
                out.append(inst)
            bb.instructions = out


def _prep_inputs(x, weight):
    """Returns list of 8 per-core input dicts."""
    npdt = ml_dtypes.bfloat16
    x = np.asarray(x, np.float32)
    w0 = np.asarray(weight, np.float32)[0]        # [CO, CI, L_OUT, K]

    wt = np.ascontiguousarray(w0.transpose(2, 3, 1, 0))   # [L_OUT, K, CI, CO]
    xt = np.ascontiguousarray(x.transpose(1, 2, 0)).astype(npdt)  # [CI, L, B]

    in_maps = []
    for m in range(N_CORES):
        hs = LP * m
        xs = xt[:, hs:hs + HALO]                  # [CI, 71, B]
        # block-diagonal stationaries tile_q [128, q, 128]
        xb = np.zeros((128, NQ, 128), npdt)
        xb[0:64, :, 0:64] = xs[:, :NQ]            # x col q      (pos 2g)
        xb[64:128, :, 64:128] = xs[:, 1:NQ + 1]   # x col q+1    (pos 2g+1)
        # w9 rows = half*64+i, col = (g*K+p)*CO+o
        a = wt[hs:hs + 62].reshape(31, 2, K, CI, CO)
        w9 = np.ascontiguousarray(a.transpose(1, 3, 0, 2, 4)) \
            .reshape(128, 31 * K * CO).astype(npdt)
        # l=62 extras in the classic pair layout
        xf = xs.astype(np.float32)                # [CI, 71, B]
        x62 = np.zeros((128, 5 * B), np.float32)
        for s in range(4):
            x62[0:64, s * B:(s + 1) * B] = xf[:, 62 + 2 * s]
            x62[64:128, s * B:(s + 1) * B] = xf[:, 63 + 2 * s]
        x62[0:64, 4 * B:] = xf[:, 70]
        wl = wt[hs + 62]                          # [K, CI, CO]
        w62 = np.zeros((128, 5 * CO), np.float32)
        w62[:, :4 * CO] = (wl[:8].reshape(4, 128, CO)
                           .transpose(1, 0, 2).reshape(128, 4 * CO))
        w62[0:64, 4 * CO:] = wl[8]
        in_maps.append({
            "xb": np.ascontiguousarray(xb).reshape(128, NQ * 128),
            "w9": w9,
            "x62": x62.astype(npdt),
            "w62": w62.astype(npdt),
        })
    return in_maps


def _decode_outputs(results):
    outs = []
    for r in results:
        v = np.asarray(r["out"]).astype(np.float32)
        # [h*64+b, g*64+o] -> out[b, o, l], l = 2g+h
        t = (v.reshape(2, 64, NGRP, CO)
             .transpose(1, 3, 2, 0)
             .reshape(B, CO, NGRP * 2)[:, :, :LP])
        outs.append(t)
    return np.concatenate(outs, axis=2).astype(np.float32)  # [B, CO, L_OUT]


_CACHED_NC = None


def kernel(x, weight):
    global _CACHED_NC
    from concourse.bass_utils import run_bass_kernel_spmd

    if _CACHED_NC is None:
        _CACHED_NC = _build_bass()
    in_maps = _prep_inputs(x, weight)
    res = run_bass_kernel_spmd(_CACHED_NC, in_maps, core_ids=list(range(N_CORES)))
    return _decode_outputs(res.results)



# revision 3
# speedup vs baseline: 1.2915x; 1.2915x over previous
"""LocallyConnected1d Bass kernel for 8 TRN2 NeuronCores — w-stationary scheme.

Problem: x [64, 64, 512] f32, weight [1, 64, 64, 504, 9] f32
         out[b, o, l] = sum_{i,k} x[b, i, l+k] * weight[0, o, i, l, k]
L_out = 504 sharded 8 x 63.  All tensors travel as bf16; host packs/unpacks.

Design (per core) — weights are the STATIONARY operand, x batch columns are
the moving operand:
  - Positions pair up as (l, l+1), l even ("pair" gp = l/2, 31 full pairs +
    the lone position 62).  The PE output tile is [ (t, o) = 128, b = 64 ]:
    t in {0,1} selects the position within the pair, o = C_out.
  - Contraction is (s', i): two consecutive x columns stacked on partitions
    (top = even col q, bottom = q+1) x C_in.  One pass covers 2 x-cols x
    2 positions = 4 kernel taps; 5 passes cover the 9-tap window of both
    positions (two corner blocks are structural zeros).
  - x ships once as [128, 36*64]: unit m = x cols (2m, 2m+1) stacked — a
    pure reshape, no duplication (passes use even-aligned column pairs).
    4608 B/partition replaces the baseline's 17920 B/partition block-diag
    x tiles; that is where the DMA-ring time goes.
  - Weights ship pre-packed as per-pass stationary tiles in PE consumption
    order: 31*5 tiles [128,128] + 5 lone tiles [128,64].
  - PSUM tiles are split so no copy ever reads a tile that later matmuls
    write (the tile framework serializes at tile granularity): pairs 0-7 /
    8-15 / 16-23 in three [128,512] banks, pairs 24-29 in [128,384],
    pair 30 and the lone position in their own [128,64] tiles.
  - Tail: ring ends are staggered so each late chunk's matmuls run inside
    the preceding chunks' DMA-completion shadow; the last chunk is the lone
    position's 5th tile alone, so the end chain is one DMA-completion lag +
    1 matmul + a [128,64] Pool copy + a [128,512] DMA + the fixed epilogue.
"""

import numpy as np
import ml_dtypes

B = 64
CI = 64
CO = 64
K = 9
L = 512
L_OUT = 504
N_CORES = 8
LP = L_OUT // N_CORES          # 63 positions per core
NPAIR = 31                     # full position pairs per core
NUNIT = 36                     # x column units (2 cols each)
W_COLS = NPAIR * 640 + 5 * 64       # 20160
X_COLS = NUNIT * 64                 # 2304
OUT_COLS = 32 * CO                  # 2048

SP, ACT, POOL = "sync", "scalar", "gpsimd"

# Input chunks in PE consumption order.
#   ("w", lo, hi): pairs [lo,hi) — 640 cols each
#   ("wt", lo_col, hi_col): raw wt column range
#   ("x", lo_u, hi_u): x units [lo,hi)
LONE0 = NPAIR * 640
CHUNKS = [
    ("w", 0, 1),                    # c0   500
    ("x", 0, 14),                   # c1   691
    ("w", 1, 3),                    # c2   987
    ("x", 14, 36),                  # c3  1086
    ("w", 3, 5),                    # c4   987
    ("w", 5, 7),                    # c5   987
    ("w", 7, 9),                    # c6   987
    ("w", 9, 11),                   # c7   987
    ("w", 11, 13),                  # c8   987
    ("w", 13, 15),                  # c9   987
    ("w", 15, 17),                  # c10  987
    ("w", 17, 19),                  # c11  987
    ("w", 19, 21),                  # c12  987
    ("w", 21, 23),                  # c13  987
    ("w", 23, 25),                  # c14  987
    ("w", 25, 27),                  # c15  987
    ("w", 27, 28),                  # c16  500
    ("w", 28, 29),                  # c17  500
    ("wt", 29 * 640, LONE0 + 256),  # c18: pairs 29-30 + lone tiles 0-3, 1184
    ("wt", LONE0 + 256, W_COLS),    # c19: lone tile 4, 500(floor)
]
# ring -> chunk indices (per-ring order = consumption order).  Ring input
# ends are near-equal; the last five chunks are spread so each one's matmul
# backlog clears inside the next chunk's completion shadow.
RING_PLAN = {
    SP: [0, 3, 6, 9, 12, 15, 19],
    ACT: [1, 4, 7, 10, 13, 18],
    POOL: [2, 5, 8, 11, 14, 16, 17],
}


def _build_bass():
    import concourse.bass as bass
    import concourse.mybir as mybir
    from concourse.tile import TileContext

    dt = mybir.dt.bfloat16
    nc = bass.Bass()

    wt_d = nc.dram_tensor("wt", [128, W_COLS], dt, kind="ExternalInput")
    xd_d = nc.dram_tensor("xd", [128, X_COLS], dt, kind="ExternalInput")
    out_d = nc.dram_tensor("out", [128, OUT_COLS], dt, kind="ExternalOutput")

    with TileContext(nc) as tc:
        with (
            tc.tile_pool(name="wc", bufs=1) as wpool,
            tc.tile_pool(name="xc", bufs=1) as xpool,
            tc.tile_pool(name="ps", bufs=1, space="PSUM") as ppool,
            tc.tile_pool(name="ob", bufs=1) as opool,
        ):
            tiles = {}
            for ci, ch in enumerate(CHUNKS):
                if ch[0] == "w":
                    tiles[ci] = wpool.tile([128, (ch[2] - ch[1]) * 640], dt,
                                           name=f"c{ci}")
                elif ch[0] == "wt":
                    tiles[ci] = wpool.tile([128, ch[2] - ch[1]], dt,
                                           name=f"c{ci}")
                else:
                    tiles[ci] = xpool.tile([128, (ch[2] - ch[1]) * 64], dt,
                                           name=f"c{ci}")

            def issue(ci):
                ch = CHUNKS[ci]
                ring = next(e for e, lst in RING_PLAN.items() if ci in lst)
                eng = getattr(nc, ring)
                if ch[0] == "w":
                    eng.dma_start(out=tiles[ci],
                                  in_=wt_d[:, ch[1] * 640:ch[2] * 640])
                elif ch[0] == "wt":
                    eng.dma_start(out=tiles[ci], in_=wt_d[:, ch[1]:ch[2]])
                else:
                    eng.dma_start(out=tiles[ci],
                                  in_=xd_d[:, ch[1] * 64:ch[2] * 64])

            maxlen = max(len(v) for v in RING_PLAN.values())
            for j in range(maxlen):
                for ring in (SP, ACT, POOL):
                    if j < len(RING_PLAN[ring]):
                        issue(RING_PLAN[ring][j])

            def wslice(gp, s):
                """lhsT tile for pair gp pass s (gp=NPAIR -> lone tile s)."""
                if gp < NPAIR:
                    col, width = gp * 640 + s * 128, 128
                else:
                    col, width = LONE0 + s * 64, 64
                for ci, ch in enumerate(CHUNKS):
                    if ch[0] == "w" and ch[1] * 640 <= col < ch[2] * 640:
                        off = col - ch[1] * 640
                        return tiles[ci][:, off:off + width]
                    if ch[0] == "wt" and ch[1] <= col < ch[2]:
                        off = col - ch[1]
                        return tiles[ci][:, off:off + width]
                raise AssertionError

            def xslice(u):
                for ci, ch in enumerate(CHUNKS):
                    if ch[0] == "x" and ch[1] <= u < ch[2]:
                        off = (u - ch[1]) * 64
                        return tiles[ci][:, off:off + 64]
                raise AssertionError

            out_sb = opool.tile([128, OUT_COLS], dt)
            # separate tiles so copies never WAR-serialize later matmuls
            pb0 = ppool.tile([128, 512], mybir.dt.float32)   # pairs 0-7
            pb1 = ppool.tile([128, 512], mybir.dt.float32)   # pairs 8-15
            pb2 = ppool.tile([128, 512], mybir.dt.float32)   # pairs 16-23
            pb3 = ppool.tile([128, 256], mybir.dt.float32)   # pairs 24-27
            pb4 = ppool.tile([128, 192], mybir.dt.float32)   # pairs 28-30
            pbL = ppool.tile([128, 64], mybir.dt.float32)    # lone position
            nc.vector.memset(pbL[64:128, :], 0.0)

            def outp_of(gp):
                if gp < 8:
                    return pb0[:, gp * 64:(gp + 1) * 64]
                if gp < 16:
                    return pb1[:, (gp - 8) * 64:(gp - 7) * 64]
                if gp < 24:
                    return pb2[:, (gp - 16) * 64:(gp - 15) * 64]
                if gp < 28:
                    return pb3[:, (gp - 24) * 64:(gp - 23) * 64]
                return pb4[:, (gp - 28) * 64:(gp - 27) * 64]

            for gp in range(NPAIR):
                outp = outp_of(gp)
                for s in range(5):
                    nc.tensor.matmul(outp, wslice(gp, s), xslice(gp + s),
                                     start=(s == 0), stop=(s == 4))
                if gp == 7:
                    nc.vector.tensor_copy(out=out_sb[:, 0:512], in_=pb0[:, :])
                elif gp == 15:
                    nc.vector.tensor_copy(out=out_sb[:, 512:1024],
                                          in_=pb1[:, :])
                    nc.sync.dma_start(out=out_d[:, 0:1024],
                                      in_=out_sb[:, 0:1024])
                elif gp == 23:
                    nc.vector.tensor_copy(out=out_sb[:, 1024:1536],
                                          in_=pb2[:, :])
                    nc.scalar.dma_start(out=out_d[:, 1024:1536],
                                        in_=out_sb[:, 1024:1536])
                elif gp == 27:
                    nc.vector.tensor_copy(out=out_sb[:, 1536:1792],
                                          in_=pb3[:, :])
                    nc.scalar.dma_start(out=out_d[:, 1536:1792],
                                        in_=out_sb[:, 1536:1792])
                elif gp == 30:
                    nc.vector.tensor_copy(out=out_sb[:, 1792:1984],
                                          in_=pb4[:, :])

            # lone position 62, fed by the tiny final w chunk
            for s in range(5):
                nc.tensor.matmul(pbL[0:64, :], wslice(NPAIR, s),
                                 xslice(NPAIR + s),
                                 start=(s == 0), stop=(s == 4))
            nc.vector.tensor_copy(out=out_sb[:, 1984:2048], in_=pbL[:, :])
            nc.sync.dma_start(out=out_d[:, 1792:2048],
                              in_=out_sb[:, 1792:2048])
    _split_multi_waits(nc, mybir)
    return nc


def _split_multi_waits(nc, mybir):
    """This walrus build encodes at most ONE sync wait per instruction;
    hoist extra waits onto single-wait NoOps (semantically identical)."""
    for f in nc.m.functions:
        for bb in f.blocks:
            out = []
            for inst in bb.instructions:
                si = inst.sync_info
                waits = list(si.on_wait) if si is not None and si.on_wait else []
                if len(waits) > 1:
                    for k, w in enumerate(waits[:-1]):
                        out.append(mybir.InstNoOp(
                            name=f"{inst.name}-wsplit{k}",
                            engine=inst.engine,
                            sync_info=mybir.SyncInfo(on_wait=[w], on_update=[]),
                            bass_nofuse=True))
                    inst.sync_info = mybir.SyncInfo(
                        on_wait=[waits[-1]],
                        on_update=list(si.on_update) if si.on_update else [])
                out.append(inst)
            bb.instructions = out


def _prep_inputs(x, weight):
    """Returns list of 8 per-core input dicts {wt, xd} (bf16)."""
    npdt = ml_dtypes.bfloat16
    x = np.asarray(x, np.float32)
    w0 = np.asarray(weight, np.float32)[0]               # [CO, CI, L_OUT, K]
    wtr = np.ascontiguousarray(w0.transpose(2, 3, 1, 0))  # [L_OUT, K, CI, CO]
    xt = np.ascontiguousarray(x.transpose(1, 2, 0))       # [CI, L, B]

    in_maps = []
    for m in range(N_CORES):
        L0 = LP * m
        # full-pair stationaries [31 pairs, 5 passes, 128 rows, 128 cols]
        arr = np.zeros((NPAIR, 5, 128, 128), np.float32)
        ls = L0 + 2 * np.arange(NPAIR)                   # t=0 positions
        for s in range(5):
            for sp in range(2):
                for t in range(2):
                    k = 2 * s + sp - t
                    if 0 <= k <= 8:
                        arr[:, s, 64 * sp:64 * sp + 64, 64 * t:64 * t + 64] \
                            = wtr[ls + t, k]
        # tile (gp, s) occupies cols [(gp*5+s)*128, +128), rows 0:128
        wt = arr.transpose(2, 0, 1, 3).reshape(128, NPAIR * 5 * 128)
        # lone tiles [5, 128 rows, 64 cols]
        lone = np.zeros((5, 128, 64), np.float32)
        for s in range(5):
            for sp in range(2):
                k = 2 * s + sp
                if k <= 8:
                    lone[s, 64 * sp:64 * sp + 64, :] = wtr[L0 + 62, k]
        wl = lone.transpose(1, 0, 2).reshape(128, 5 * 64)
        wt_full = np.concatenate([wt, wl], axis=1).astype(npdt)

        # x units [128, 36*64]: unit u rows 0:64 = x col L0+2u, rows 64:128 =
        # x col L0+2u+1 (zero past L-1)
        xd = np.zeros((128, NUNIT, B), np.float32)
        for u in range(NUNIT):
            c0, c1 = L0 + 2 * u, L0 + 2 * u + 1
            if c0 < L:
                xd[0:64, u] = xt[:, c0]
            if c1 < L:
                xd[64:128, u] = xt[:, c1]
        xd = xd.reshape(128, NUNIT * B).astype(npdt)

        in_maps.append({"wt": np.ascontiguousarray(wt_full),
                        "xd": np.ascontiguousarray(xd)})
    return in_maps


def _decode_outputs(results):
    outs = []
    for r in results:
        v = np.asarray(r["out"]).astype(np.float32)      # [128, 2048]
        # col block 64*gp holds pair gp as [t*64+o rows, b cols]; block 31
        # holds the lone position (t=0 only)
        blk = v.reshape(2, CO, 32, B)                    # [t, o, gp, b]
        t = blk.transpose(3, 1, 2, 0).reshape(B, CO, 64)  # [b, o, l=2gp+t]
        outs.append(t[:, :, :LP])
    return np.concatenate(outs, axis=2).astype(np.float32)


_CACHED_NC = None


def kernel(x, weight):
    global _CACHED_NC
    from concourse.bass_utils import run_bass_kernel_spmd

    if _CACHED_NC is None:
        _CACHED_NC = _build_bass()
    in_maps = _prep_inputs(x, weight)
    res = run_bass_kernel_spmd(_CACHED_NC, in_maps, core_ids=list(range(N_CORES)))
    return _decode_outputs(res.results)


# revision 4
# speedup vs baseline: 1.3034x; 1.0092x over previous
"""LocallyConnected1d Bass kernel for 8 TRN2 NeuronCores — w-stationary scheme.

Problem: x [64, 64, 512] f32, weight [1, 64, 64, 504, 9] f32
         out[b, o, l] = sum_{i,k} x[b, i, l+k] * weight[0, o, i, l, k]
L_out = 504 sharded 8 x 63.  All tensors travel as bf16; host packs/unpacks.

Design (per core) — weights are the STATIONARY operand, x batch columns are
the moving operand:
  - Positions pair up as (l, l+1), l even ("pair" gp = l/2, 31 full pairs +
    the lone position 62).  The PE output tile is [ (t, o) = 128, b = 64 ]:
    t in {0,1} selects the position within the pair, o = C_out.
  - Contraction is (s', i): two consecutive x columns stacked on partitions
    (top = even col q, bottom = q+1) x C_in.  One pass covers 2 x-cols x
    2 positions = 4 kernel taps; 5 passes cover the 9-tap window of both
    positions (two corner blocks are structural zeros).
  - x ships once as [128, 36*64]: unit m = x cols (2m, 2m+1) stacked — a
    pure reshape, no duplication (passes use even-aligned column pairs).
    4608 B/partition replaces the baseline's 17920 B/partition block-diag
    x tiles; that is where the DMA-ring time goes.
  - Weights ship pre-packed as per-pass stationary tiles in PE consumption
    order: 31*5 tiles [128,128] + 5 lone tiles [128,64].
  - PSUM tiles are split so no copy ever reads a tile that later matmuls
    write (the tile framework serializes at tile granularity): pairs 0-7 /
    8-15 / 16-23 in three [128,512] banks, pairs 24-29 in [128,384],
    pair 30 and the lone position in their own [128,64] tiles.
  - Tail: ring ends are staggered so each late chunk's matmuls run inside
    the preceding chunks' DMA-completion shadow; the last chunk is the lone
    position's 5th tile alone, so the end chain is one DMA-completion lag +
    1 matmul + a [128,64] Pool copy + a [128,512] DMA + the fixed epilogue.
"""

import numpy as np
import ml_dtypes

B = 64
CI = 64
CO = 64
K = 9
L = 512
L_OUT = 504
N_CORES = 8
LP = L_OUT // N_CORES          # 63 positions per core
NPAIR = 31                     # full position pairs per core
NUNIT = 36                     # x column units (2 cols each)
W_COLS = NPAIR * 640 + 5 * 64       # 20160
X_COLS = NUNIT * 64                 # 2304
OUT_COLS = 32 * CO                  # 2048

SP, ACT, POOL = "sync", "scalar", "gpsimd"

# Input chunks in PE consumption order.
#   ("w", lo, hi): pairs [lo,hi) — 640 cols each
#   ("wt", lo_col, hi_col): raw wt column range
#   ("x", lo_u, hi_u): x units [lo,hi)
LONE0 = NPAIR * 640
CHUNKS = [
    ("w", 0, 1),                    # c0   500
    ("x", 0, 14),                   # c1   691
    ("w", 1, 3),                    # c2   987
    ("x", 14, 36),                  # c3  1086
    ("w", 3, 5),                    # c4   987
    ("w", 5, 7),                    # c5   987
    ("w", 7, 9),                    # c6   987
    ("w", 9, 11),                   # c7   987
    ("w", 11, 13),                  # c8   987
    ("w", 13, 15),                  # c9   987
    ("w", 15, 17),                  # c10  987
    ("w", 17, 19),                  # c11  987
    ("w", 19, 21),                  # c12  987
    ("w", 21, 23),                  # c13  987
    ("w", 23, 25),                  # c14  987
    ("w", 25, 27),                  # c15  987
    ("w", 27, 28),                  # c16  500
    ("w", 28, 29),                  # c17  500
    ("wt", 29 * 640, LONE0 + 256),  # c18: pairs 29-30 + lone tiles 0-3, 1184
    ("wt", LONE0 + 256, W_COLS),    # c19: lone tile 4, 500(floor)
]
# ring -> chunk indices (per-ring order = consumption order).  Ring input
# ends are near-equal; the last five chunks are spread so each one's matmul
# backlog clears inside the next chunk's completion shadow.
RING_PLAN = {
    SP: [0, 3, 6, 9, 12, 15, 19],
    ACT: [1, 4, 7, 10, 13, 18],
    POOL: [2, 5, 8, 11, 14, 16, 17],
}


def _build_bass():
    import concourse.bass as bass
    import concourse.mybir as mybir
    from concourse.tile import TileContext

    dt = mybir.dt.bfloat16
    nc = bass.Bass()

    wt_d = nc.dram_tensor("wt", [128, W_COLS], dt, kind="ExternalInput")
    xd_d = nc.dram_tensor("xd", [128, X_COLS], dt, kind="ExternalInput")
    out_d = nc.dram_tensor("out", [128, OUT_COLS], dt, kind="ExternalOutput")

    with TileContext(nc) as tc:
        with (
            tc.tile_pool(name="wc", bufs=1) as wpool,
            tc.tile_pool(name="xc", bufs=1) as xpool,
            tc.tile_pool(name="ps", bufs=1, space="PSUM") as ppool,
            tc.tile_pool(name="ob", bufs=1) as opool,
        ):
            tiles = {}
            for ci, ch in enumerate(CHUNKS):
                if ch[0] == "w":
                    tiles[ci] = wpool.tile([128, (ch[2] - ch[1]) * 640], dt,
                                           name=f"c{ci}")
                elif ch[0] == "wt":
                    tiles[ci] = wpool.tile([128, ch[2] - ch[1]], dt,
                                           name=f"c{ci}")
                else:
                    tiles[ci] = xpool.tile([128, (ch[2] - ch[1]) * 64], dt,
                                           name=f"c{ci}")

            def issue(ci):
                ch = CHUNKS[ci]
                ring = next(e for e, lst in RING_PLAN.items() if ci in lst)
                eng = getattr(nc, ring)
                if ch[0] == "w":
                    eng.dma_start(out=tiles[ci],
                                  in_=wt_d[:, ch[1] * 640:ch[2] * 640])
                elif ch[0] == "wt":
                    eng.dma_start(out=tiles[ci], in_=wt_d[:, ch[1]:ch[2]])
                else:
                    eng.dma_start(out=tiles[ci],
                                  in_=xd_d[:, ch[1] * 64:ch[2] * 64])

            maxlen = max(len(v) for v in RING_PLAN.values())
            for j in range(maxlen):
                for ring in (SP, ACT, POOL):
                    if j < len(RING_PLAN[ring]):
                        issue(RING_PLAN[ring][j])

            def wslice(gp, s):
                """lhsT tile for pair gp pass s (gp=NPAIR -> lone tile s)."""
                if gp < NPAIR:
                    col, width = gp * 640 + s * 128, 128
                else:
                    col, width = LONE0 + s * 64, 64
                for ci, ch in enumerate(CHUNKS):
                    if ch[0] == "w" and ch[1] * 640 <= col < ch[2] * 640:
                        off = col - ch[1] * 640
                        return tiles[ci][:, off:off + width]
                    if ch[0] == "wt" and ch[1] <= col < ch[2]:
                        off = col - ch[1]
                        return tiles[ci][:, off:off + width]
                raise AssertionError

            def xslice(u):
                for ci, ch in enumerate(CHUNKS):
                    if ch[0] == "x" and ch[1] <= u < ch[2]:
                        off = (u - ch[1]) * 64
                        return tiles[ci][:, off:off + 64]
                raise AssertionError

            out_sb = opool.tile([128, OUT_COLS], dt)
            # separate tiles so copies never WAR-serialize later matmuls
            pb0 = ppool.tile([128, 512], mybir.dt.float32)   # pairs 0-7
            pb1 = ppool.tile([128, 512], mybir.dt.float32)   # pairs 8-15
            pb2 = ppool.tile([128, 512], mybir.dt.float32)   # pairs 16-23
            pb3 = ppool.tile([128, 256], mybir.dt.float32)   # pairs 24-27
            # pairs 28-30 + lone share one tile -> one final copy/wait
            pb4 = ppool.tile([128, 256], mybir.dt.float32)
            nc.vector.memset(pb4[64:128, 192:256], 0.0)

            def outp_of(gp):
                if gp < 8:
                    return pb0[:, gp * 64:(gp + 1) * 64]
                if gp < 16:
                    return pb1[:, (gp - 8) * 64:(gp - 7) * 64]
                if gp < 24:
                    return pb2[:, (gp - 16) * 64:(gp - 15) * 64]
                if gp < 28:
                    return pb3[:, (gp - 24) * 64:(gp - 23) * 64]
                return pb4[:, (gp - 28) * 64:(gp - 27) * 64]

            for gp in range(NPAIR):
                outp = outp_of(gp)
                for s in range(5):
                    nc.tensor.matmul(outp, wslice(gp, s), xslice(gp + s),
                                     start=(s == 0), stop=(s == 4))
                if gp == 7:
                    nc.vector.tensor_copy(out=out_sb[:, 0:512], in_=pb0[:, :])
                elif gp == 15:
                    nc.vector.tensor_copy(out=out_sb[:, 512:1024],
                                          in_=pb1[:, :])
                    nc.sync.dma_start(out=out_d[:, 0:1024],
                                      in_=out_sb[:, 0:1024])
                elif gp == 23:
                    nc.vector.tensor_copy(out=out_sb[:, 1024:1536],
                                          in_=pb2[:, :])
                    nc.scalar.dma_start(out=out_d[:, 1024:1536],
                                        in_=out_sb[:, 1024:1536])
                elif gp == 27:
                    nc.vector.tensor_copy(out=out_sb[:, 1536:1792],
                                          in_=pb3[:, :])
                    nc.scalar.dma_start(out=out_d[:, 1536:1792],
                                        in_=out_sb[:, 1536:1792])


            # lone position 62, fed by the tiny final w chunk
            for s in range(5):
                nc.tensor.matmul(pb4[0:64, 192:256], wslice(NPAIR, s),
                                 xslice(NPAIR + s),
                                 start=(s == 0), stop=(s == 4))
            nc.vector.tensor_copy(out=out_sb[:, 1792:2048],
                                  in_=pb4[:, :])
            nc.sync.dma_start(out=out_d[:, 1792:2048],
                              in_=out_sb[:, 1792:2048])
    _split_multi_waits(nc, mybir)
    return nc


def _split_multi_waits(nc, mybir):
    """This walrus build encodes at most ONE sync wait per instruction;
    hoist extra waits onto single-wait NoOps (semantically identical)."""
    for f in nc.m.functions:
        for bb in f.blocks:
            out = []
            for inst in bb.instructions:
                si = inst.sync_info
                waits = list(si.on_wait) if si is not None and si.on_wait else []
                if len(waits) > 1:
                    for k, w in enumerate(waits[:-1]):
                        out.append(mybir.InstNoOp(
                            name=f"{inst.name}-wsplit{k}",
                            engine=inst.engine,
                            sync_info=mybir.SyncInfo(on_wait=[w], on_update=[]),
                            bass_nofuse=True))
                    inst.sync_info = mybir.SyncInfo(
                        on_wait=[waits[-1]],
                        on_update=list(si.on_update) if si.on_update else [])
                out.append(inst)
            bb.instructions = out


def _prep_inputs(x, weight):
    """Returns list of 8 per-core input dicts {wt, xd} (bf16)."""
    npdt = ml_dtypes.bfloat16
    x = np.asarray(x, np.float32)
    w0 = np.asarray(weight, np.float32)[0]               # [CO, CI, L_OUT, K]
    wtr = np.ascontiguousarray(w0.transpose(2, 3, 1, 0))  # [L_OUT, K, CI, CO]
    xt = np.ascontiguousarray(x.transpose(1, 2, 0))       # [CI, L, B]

    in_maps = []
    for m in range(N_CORES):
        L0 = LP * m
        # full-pair stationaries [31 pairs, 5 passes, 128 rows, 128 cols]
        arr = np.zeros((NPAIR, 5, 128, 128), np.float32)
        ls = L0 + 2 * np.arange(NPAIR)                   # t=0 positions
        for s in range(5):
            for sp in range(2):
                for t in range(2):
                    k = 2 * s + sp - t
                    if 0 <= k <= 8:
                        arr[:, s, 64 * sp:64 * sp + 64, 64 * t:64 * t + 64] \
                            = wtr[ls + t, k]
        # tile (gp, s) occupies cols [(gp*5+s)*128, +128), rows 0:128
        wt = arr.transpose(2, 0, 1, 3).reshape(128, NPAIR * 5 * 128)
        # lone tiles [5, 128 rows, 64 cols]
        lone = np.zeros((5, 128, 64), np.float32)
        for s in range(5):
            for sp in range(2):
                k = 2 * s + sp
                if k <= 8:
                    lone[s, 64 * sp:64 * sp + 64, :] = wtr[L0 + 62, k]
        wl = lone.transpose(1, 0, 2).reshape(128, 5 * 64)
        wt_full = np.concatenate([wt, wl], axis=1).astype(npdt)

        # x units [128, 36*64]: unit u rows 0:64 = x col L0+2u, rows 64:128 =
        # x col L0+2u+1 (zero past L-1)
        xd = np.zeros((128, NUNIT, B), np.float32)
        for u in range(NUNIT):
            c0, c1 = L0 + 2 * u, L0 + 2 * u + 1
            if c0 < L:
                xd[0:64, u] = xt[:, c0]
            if c1 < L:
                xd[64:128, u] = xt[:, c1]
        xd = xd.reshape(128, NUNIT * B).astype(npdt)

        in_maps.append({"wt": np.ascontiguousarray(wt_full),
                        "xd": np.ascontiguousarray(xd)})
    return in_maps


def _decode_outputs(results):
    outs = []
    for r in results:
        v = np.asarray(r["out"]).astype(np.float32)      # [128, 2048]
        # col block 64*gp holds pair gp as [t*64+o rows, b cols]; block 31
        # holds the lone position (t=0 only)
        blk = v.reshape(2, CO, 32, B)                    # [t, o, gp, b]
        t = blk.transpose(3, 1, 2, 0).reshape(B, CO, 64)  # [b, o, l=2gp+t]
        outs.append(t[:, :, :LP])
    return np.concatenate(outs, axis=2).astype(np.float32)


_CACHED_NC = None


def kernel(x, weight):
    global _CACHED_NC
    from concourse.bass_utils import run_bass_kernel_spmd

    if _CACHED_NC is None:
        _CACHED_NC = _build_bass()
    in_maps = _prep_inputs(x, weight)
    res = run_bass_kernel_spmd(_CACHED_NC, in_maps, core_ids=list(range(N_CORES)))
    return _decode_outputs(res.results)
